# revision 4
# baseline (speedup 1.0000x reference)
"""Trainium2 Bass kernel for the CMPO2/GTN MPS-chain contraction (v2).

Computation (see harness reference): for each sample s,
    v0  = psi_first^T x[s,0]                                  [D]
    v_{i+1}[e] = sum_{d,p} v_i[d] psi_mid[i][d,e,p] x[s,1+i,p]   (62 steps)
    out_vec[s] = sum_{d,p} v[d] psi_last[d,p,o] x[s,63,p]     [B, O]
    out[s] = c * out_vec[s]   with c the (batch-independent) phi-chain scalar.

v2 changes vs the HBM-streaming baseline (796us, HBM-bound at 282MB/core):
  * phi chain scalar c is batch-independent -> computed on the HOST and
    folded into the final output scale (no device work, frees a PSUM bank).
  * Only NSG of 8 x-broadcast groups are streamed pre-broadcast from HBM;
    the other NBG groups are produced ON-CHIP by exact replication matmuls
    (stationary = 0/1 pattern, K=32) on otherwise-stalled PE slots, using a
    quad-replicated compact x (x4) as the moving operand, row-tiled across
    the 4 PE array quadrants.  Cuts HBM from ~4.5 to ~3.0 MB/step.
  * c-outer matmul ordering (stationary reuse), fatter DVE multiplies, one
    u-product group offloaded to GpSimd, v2 column-dup replaced by a
    stride-0 broadcast access pattern, A column-dup done by AP (no copy).
"""

import numpy as np

N_CORES = 8
B, Q, P, D, L, O = 8192, 64, 32, 64, 64, 10
BL = B // N_CORES          # batch per core
TN = 512                   # matmul free-dim tile (one PSUM bank of fp32)
NT = BL // TN              # N tiles per batch shard
NCH = (D * P) // 128       # 16 K-chunks of 128 over (p,d)
NG = 8                     # chunk pairs (2 chunks each)
NMID = L - 2               # 62 middle sites
SH_LAST = 6                # 2^SH_LAST folded into psi_last (fp16 subnormal avoidance)
VBAND = 16.0               # target |v| band for the scale schedule

# ---- tuning knobs ----
NBG = 3                    # groups produced via PE broadcast (chunks 16-2*NBG..15)
NSG = NG - NBG             # groups streamed pre-broadcast from HBM
ROW_TILED = True           # put bcast matmuls on PE array quadrants
GP_GROUPS = (4,)           # u-mul groups computed on gpsimd instead of DVE
STRIDE0_V2 = True          # v2 [128,BL] read with stride-0 q-dim (skip col-dup)
STRIDE0_LHS = False        # A stationary column-dup via stride-0 AP (BIR rejects)

# global row r in 0..2047 of u/(A rows): p = 2*(r//128) + (r%128)//64 ; d = r%64
_P_IDX = np.repeat(np.arange(P), D)          # [2048]
_D_IDX = np.tile(np.arange(D), P)            # [2048]

_cached = {}


def _ensure_path():
    import sys
    for p in ("/opt/trn_rl_repo", "/root/.axon_site/_ro/trn_rl_repo"):
        try:
            import concourse  # noqa: F401
            return
        except Exception:
            if p not in sys.path:
                sys.path.insert(0, p)
    import concourse  # noqa: F401


def _build_program():
    """Build + compile the Bass/Tile program (shared by all 8 cores)."""
    _ensure_path()
    from concourse import bacc, tile, mybir

    dt = mybir.dt
    nc = bacc.Bacc(
        "TRN2",
        target_bir_lowering=False,
        debug=False,
        enable_asserts=False,
        num_devices=N_CORES,
    )

    a_d = nc.dram_tensor("a_w", [NMID, 128, NCH * D], dt.float16, kind="ExternalInput").ap()
    xb_d = nc.dram_tensor("xb", [NMID + 1, 128, NSG * 2 * BL], dt.float16, kind="ExternalInput").ap()
    x0_d = nc.dram_tensor("x0", [P, BL], dt.float16, kind="ExternalInput").ap()
    pf_d = nc.dram_tensor("pf", [P, 128], dt.float16, kind="ExternalInput").ap()
    pl_d = nc.dram_tensor("pl", [128, NCH * O], dt.float16, kind="ExternalInput").ap()
    out_d = nc.dram_tensor("out", [O, BL], dt.float32, kind="ExternalOutput").ap()
    if NBG:
        x4_d = nc.dram_tensor("x4", [NMID + 1, 128, BL], dt.float16, kind="ExternalInput").ap()
        rep_d = nc.dram_tensor("rep", [128, ((2 * NBG + 3) // 4) * 128], dt.float16, kind="ExternalInput").ap()

    with tile.TileContext(nc) as tc:
        with tc.tile_pool(name="const", bufs=1) as cpool, \
             tc.tile_pool(name="aw", bufs=3) as apool, \
             tc.tile_pool(name="awd", bufs=2) as adpool, \
             tc.tile_pool(name="xbp", bufs=2) as xbpool, \
             tc.tile_pool(name="x4p", bufs=3) as x4pool, \
             tc.tile_pool(name="bcxp", bufs=2 * max(NBG, 1)) as bcxpool, \
             tc.tile_pool(name="vrp", bufs=2) as vrpool, \
             tc.tile_pool(name="up", bufs=10) as upool, \
             tc.tile_pool(name="misc", bufs=1) as mpool, \
             tc.tile_pool(name="pvp", bufs=4, space="PSUM") as pvpool, \
             tc.tile_pool(name="bcp", bufs=2 * min(NBG, 2), space="PSUM") as bcpool:

            # --- constants / per-core inputs resident in SBUF ---
            pf_sb = cpool.tile([P, 128], dt.float16, name="pf_sb")
            nc.sync.dma_start(out=pf_sb, in_=pf_d)
            pl_sb = cpool.tile([128, NCH * O], dt.float16, name="pl_sb")
            nc.sync.dma_start(out=pl_sb, in_=pl_d)
            x0_sb = cpool.tile([P, BL], dt.float16, name="x0_sb")
            nc.sync.dma_start(out=x0_sb, in_=x0_d)
            if NBG:
                rep_sb = cpool.tile([128, ((2 * NBG + 3) // 4) * 128], dt.float16, name="rep_sb")
                nc.sync.dma_start(out=rep_sb, in_=rep_d)

            # --- v0 = [psi_first^T | psi_first^T] @ x0 -> [v0 ; v0] ---
            pv_cur = []
            for t in range(NT):
                pv = pvpool.tile([128, TN], dt.float32, name="pv")
                nc.tensor.matmul(out=pv, lhsT=pf_sb,
                                 rhs=x0_sb[:, t * TN:(t + 1) * TN],
                                 start=True, stop=True)
                pv_cur.append(pv)

            def emit_bcast(i):
                """PE replication matmuls producing bcast xb tiles for step i."""
                x4_sb = x4pool.tile([128, BL], dt.float16, name="x4_sb")
                nc.sync.dma_start(out=x4_sb, in_=x4_d[i])
                tiles = []
                for g in range(NBG):
                    bcx = bcxpool.tile([128, 2 * BL], dt.float16, name="bcx")
                    tiles.append(bcx)
                for j in range(2 * NBG):
                    g, q = j // 2, j % 2
                    if ROW_TILED:
                        k = j % 4
                    else:
                        k = 0
                    jq = j // 4 if ROW_TILED else 0
                    lhs = rep_sb[32 * k:32 * (k + 1), jq * 128:(jq + 1) * 128]
                    bps = bcpool.tile([128, TN], dt.float32, name="bps")
                    bps2 = bcpool.tile([128, TN], dt.float32, name="bps")
                    for t, ps in enumerate((bps, bps2)):
                        nc.tensor.matmul(
                            out=ps, lhsT=lhs,
                            rhs=x4_sb[32 * k:32 * (k + 1), t * TN:(t + 1) * TN],
                            start=True, stop=True,
                            tile_position=(32 * k, 0) if ROW_TILED else None)
                    for t, ps in enumerate((bps, bps2)):
                        nc.scalar.copy(
                            out=tiles[g][:, q * BL + t * TN: q * BL + (t + 1) * TN],
                            in_=ps)
                return tiles

            bcx_cur = emit_bcast(0) if NBG else []

            for i in range(NMID + 1):
                last = (i == NMID)
                # evacuate [vT; vT] into v2 (scalar engine, PSUM -> SBUF fp16)
                if STRIDE0_V2:
                    v2 = vrpool.tile([128, BL], dt.float16, name="v2")
                    for t in range(NT):
                        nc.scalar.copy(out=v2[:, t * TN:(t + 1) * TN], in_=pv_cur[t])
                    v2b = v2.unsqueeze(1).broadcast_to([128, 2, BL])
                else:
                    v2 = vrpool.tile([128, 2 * BL], dt.float16, name="v2")
                    for t in range(NT):
                        nc.scalar.copy(out=v2[:, t * TN:(t + 1) * TN], in_=pv_cur[t])
                    nc.scalar.copy(out=v2[:, BL:2 * BL], in_=v2[:, 0:BL])
                    v2b = v2.rearrange("p (q s) -> p q s", q=2)

                # streamed x-broadcast for this step: two ~1.3MB DMAs
                xq_sb = xbpool.tile([128, NSG * 2 * BL], dt.float16, name="xq_sb")
                half = NSG * BL
                nc.sync.dma_start(out=xq_sb[:, :half], in_=xb_d[i, :, :half])
                nc.scalar.dma_start(out=xq_sb[:, half:], in_=xb_d[i, :, half:])

                if not last:
                    a_raw = apool.tile([128, NCH * D], dt.float16, name="a_raw")
                    nc.scalar.dma_start(out=a_raw, in_=a_d[i])
                    if STRIDE0_LHS:
                        # lhsT chunk c: [128, 128] = A cols duplicated via AP
                        a_lhs = a_raw.rearrange("p (c e) -> p c e", c=NCH) \
                                     .unsqueeze(2).broadcast_to([128, NCH, 2, D])
                    else:
                        a_sb = adpool.tile([128, NCH * 128], dt.float16, name="a_sb")
                        av = a_sb.rearrange("p (c j e) -> p c j e", c=NCH, j=2, e=D)
                        ar = a_raw.rearrange("p (c e) -> p c e", c=NCH)
                        nc.scalar.copy(out=av[:, :, 0, :], in_=ar)
                        nc.scalar.copy(out=av[:, :, 1, :], in_=ar)
                        a_lhs = a_sb.rearrange("p (c m) -> p c m", c=NCH)
                    pv_nxt = []
                    for t in range(NT):
                        pv = pvpool.tile([128, TN], dt.float32, name="pv")
                        pv_nxt.append(pv)
                else:
                    po = []
                    for t in range(NT):
                        p_o = pvpool.tile([O, TN], dt.float32, name="pv")
                        po.append(p_o)

                # u-products: us[g] = v2 (q-broadcast) * xb group tile
                us = []
                for g in range(NG):
                    u2 = upool.tile([128, 2 * BL], dt.float16, name="u2")
                    if g < NSG:
                        xbg = xq_sb[:, g * 2 * BL:(g + 1) * 2 * BL]
                    else:
                        xbg = bcx_cur[g - NSG]
                    eng = nc.gpsimd if g in GP_GROUPS else nc.vector
                    eng.tensor_mul(u2.rearrange("p (q s) -> p q s", q=2), v2b,
                                   xbg.rearrange("p (q s) -> p q s", q=2))
                    us.append(u2)

                # main contraction, chunk-outer (stationary reuse across t)
                out_ps = po if last else pv_nxt
                for c in range(NCH):
                    g, q = c // 2, c % 2
                    if last:
                        lhs = pl_sb[:, c * O:(c + 1) * O]
                    elif STRIDE0_LHS:
                        lhs = a_lhs[:, c]
                    else:
                        lhs = a_lhs[:, c]
                    for t in range(NT):
                        nc.tensor.matmul(
                            out=out_ps[t],
                            lhsT=lhs,
                            rhs=us[g][:, q * BL + t * TN: q * BL + (t + 1) * TN],
                            start=(c == 0), stop=(c == NCH - 1))

                # broadcast xb tiles for step i+1 (PE + scalar evac)
                if NBG and not last:
                    bcx_cur = emit_bcast(i + 1)
                if not last:
                    pv_cur = pv_nxt

            # --- final scale by host-computed phi scalar (folded constant) ---
            out_sb = mpool.tile([O, BL], dt.float32, name="out_sb")
            sc_d = nc.dram_tensor("fscale", [1, 1], dt.float32, kind="ExternalInput").ap()
            sc_sb = mpool.tile([1, 1], dt.float32, name="sc_sb")
            nc.sync.dma_start(out=sc_sb, in_=sc_d)
            sc10 = mpool.tile([O, 1], dt.float32, name="sc10")
            nc.gpsimd.partition_broadcast(sc10, sc_sb)
            for t in range(NT):
                nc.scalar.mul(out=out_sb[:, t * TN:(t + 1) * TN], in_=po[t], mul=sc10)
            nc.sync.dma_start(out=out_d, in_=out_sb)

    nc.compile()
    return nc


def _scale_schedule(x, psi_first, psi_mid, nsub=128):
    """Static per-step power-of-2 downscales keeping |v| in a small band."""
    xs = np.asarray(x[:nsub], np.float32)
    v = xs[:, 0] @ np.asarray(psi_first, np.float32).T
    ks = []
    for i in range(NMID):
        A = np.asarray(psi_mid[i], np.float32)            # [d, e, p]
        xi = xs[:, 1 + i]                                  # [s, p]
        u = np.einsum('sd,sp->sdp', v, xi).reshape(nsub, D * P)
        v = u @ A.transpose(0, 2, 1).reshape(D * P, D)
        vm = float(np.abs(v).max())
        k = 0
        while vm * 2.0 ** (-k) > VBAND:
            k += 1
        ks.append(k)
        v = v * 2.0 ** (-k)
    return ks


def kernel(x, psi_first, psi_mid, psi_last, phi_first, phi_mid, phi_last):
    _ensure_path()
    from concourse import bass_utils

    f16 = np.float16
    x = np.asarray(x, np.float32)
    psi_first = np.asarray(psi_first, np.float32)
    psi_mid = np.asarray(psi_mid, np.float32)
    psi_last = np.asarray(psi_last, np.float32)
    phi_first = np.asarray(phi_first, np.float64)
    phi_mid = np.asarray(phi_mid, np.float64)
    phi_last = np.asarray(phi_last, np.float64)

    if "nc" not in _cached:
        _cached["nc"] = _build_program()
    nc = _cached["nc"]

    ks = _scale_schedule(x, psi_first, psi_mid)

    # --- host phi chain: batch-independent scalar c ---
    w = phi_first[:, 0].copy()
    for i in range(NMID):
        w = w @ phi_mid[i, :, :, 1 + i]
    c_phi = float(w @ phi_last[:, Q - 1])

    # --- shared weight-side arrays ---
    scales = (2.0 ** -np.asarray(ks, np.float64)).astype(np.float32)
    # A2[i, r, e] = psi_mid[i, d(r), e, p(r)] * s_i  -> [62, 2048, 64]
    A2 = psi_mid.transpose(0, 1, 3, 2)[:, _D_IDX, _P_IDX, :]        # [62, 2048, 64]
    A2 = A2 * scales[:, None, None]
    A2c = A2.reshape(NMID, NCH, 128, D)
    a_host = np.ascontiguousarray(
        A2c.transpose(0, 2, 1, 3).reshape(NMID, 128, NCH * D)
    ).astype(f16)

    pf_host = np.concatenate([psi_first.T, psi_first.T], axis=1).astype(f16)  # [32, 128]

    # pl2[r, o] = psi_last[d(r), p(r), o] * 2^SH -> chunked [128, 16*O]
    pl2 = (psi_last * (2.0 ** SH_LAST))[_D_IDX, _P_IDX, :]          # [2048, O]
    pl_host = np.ascontiguousarray(
        pl2.reshape(NCH, 128, O).transpose(1, 0, 2).reshape(128, NCH * O)
    ).astype(f16)

    fscale_host = np.array(
        [[c_phi * 2.0 ** (sum(ks) - SH_LAST)]], dtype=np.float32)

    if NBG:
        # rep[32k + p, jq*128 + m] = 1 if p == 2*c + m//64 (c = global bcast chunk)
        nrq = ((2 * NBG + 3) // 4)
        rep_host = np.zeros((128, nrq * 128), f16)
        for j in range(2 * NBG):
            c = NCH - 2 * NBG + j
            k = (j % 4) if ROW_TILED else 0
            jq = (j // 4) if ROW_TILED else 0
            for m in range(128):
                p = 2 * c + m // 64
                rep_host[32 * k + p, jq * 128 + m] = 1.0

    # --- per-core batch shards ---
    xt = x.transpose(1, 2, 0).astype(f16)         # [Q, P, B]
    x0_all = xt[0]                                # [P, B]
    ridx = np.arange(128) // 64                   # [128] -> 0/1 within chunk
    in_maps = []
    for ci in range(N_CORES):
        sl = slice(ci * BL, (ci + 1) * BL)
        xs = np.ascontiguousarray(xt[1:, :, sl])            # [63, P, BL]
        # streamed groups: xbg[i, r, g*2BL + q*BL + s] = xs[i, 2*(2g+q)+r//64, s]
        xbg = np.empty((NMID + 1, 128, NSG, 2, BL), f16)
        for g in range(NSG):
            for q in range(2):
                xbg[:, :, g, q, :] = xs[:, 2 * (2 * g + q) + ridx, :]
        xbg = xbg.reshape(NMID + 1, 128, NSG * 2 * BL)
        m = {
            "a_w": a_host,
            "xb": np.ascontiguousarray(xbg),
            "x0": np.ascontiguousarray(x0_all[:, sl]),
            "pf": pf_host,
            "pl": pl_host,
            "fscale": fscale_host,
        }
        if NBG:
            m["x4"] = np.ascontiguousarray(np.tile(xs, (1, 4, 1)))   # [63, 128, BL]
            m["rep"] = rep_host
        in_maps.append(m)

    res = bass_utils.run_bass_kernel_spmd(nc, in_maps, core_ids=list(range(N_CORES)))
    _cached["in_maps"] = in_maps

    out = np.empty((B, O), np.float32)
    for ci in range(N_CORES):
        out[ci * BL:(ci + 1) * BL, :] = res.results[ci]["out"].T
    return out


# revision 11
# speedup vs baseline: 1.0302x; 1.0302x over previous
"""Trainium2 Bass kernel for the CMPO2/GTN MPS-chain contraction (v2).

Computation (see harness reference): for each sample s,
    v0  = psi_first^T x[s,0]                                  [D]
    v_{i+1}[e] = sum_{d,p} v_i[d] psi_mid[i][d,e,p] x[s,1+i,p]   (62 steps)
    out_vec[s] = sum_{d,p} v[d] psi_last[d,p,o] x[s,63,p]     [B, O]
    out[s] = c * out_vec[s]   with c the (batch-independent) phi-chain scalar.

v2 changes vs the HBM-streaming baseline (796us, HBM-bound at 282MB/core):
  * phi chain scalar c is batch-independent -> computed on the HOST and
    folded into the final output scale (no device work, frees a PSUM bank).
  * Only NSG of 8 x-broadcast groups are streamed pre-broadcast from HBM;
    the other NBG groups are produced ON-CHIP by exact replication matmuls
    (stationary = 0/1 pattern, K=32) on otherwise-stalled PE slots, using a
    quad-replicated compact x (x4) as the moving operand, row-tiled across
    the 4 PE array quadrants.  Cuts HBM from ~4.5 to ~3.0 MB/step.
  * c-outer matmul ordering (stationary reuse), fatter DVE multiplies, one
    u-product group offloaded to GpSimd, v2 column-dup replaced by a
    stride-0 broadcast access pattern, A column-dup done by AP (no copy).
"""

import numpy as np

N_CORES = 8
B, Q, P, D, L, O = 8192, 64, 32, 64, 64, 10
BL = B // N_CORES          # batch per core
TN = 512                   # matmul free-dim tile (one PSUM bank of fp32)
NT = BL // TN              # N tiles per batch shard
NCH = (D * P) // 128       # 16 K-chunks of 128 over (p,d)
NG = 8                     # chunk pairs (2 chunks each)
NMID = L - 2               # 62 middle sites
SH_LAST = 6                # 2^SH_LAST folded into psi_last (fp16 subnormal avoidance)
VBAND = 16.0               # target |v| band for the scale schedule

# ---- tuning knobs ----
NBG = 3                    # groups produced via PE broadcast (chunks 16-2*NBG..15)
NSG = NG - NBG             # groups streamed pre-broadcast from HBM
ROW_TILED = True           # put bcast matmuls on PE array quadrants
GP_GROUPS = (4,)           # u-mul groups computed on gpsimd instead of DVE
STRIDE0_V2 = False         # stride-0 q-dim drops DVE to 1x mode; col-dup instead
STRIDE0_LHS = False        # A stationary column-dup via stride-0 AP (BIR rejects)

# global row r in 0..2047 of u/(A rows): p = 2*(r//128) + (r%128)//64 ; d = r%64
_P_IDX = np.repeat(np.arange(P), D)          # [2048]
_D_IDX = np.tile(np.arange(D), P)            # [2048]

_cached = {}


def _ensure_path():
    import sys
    for p in ("/opt/trn_rl_repo", "/root/.axon_site/_ro/trn_rl_repo"):
        try:
            import concourse  # noqa: F401
            return
        except Exception:
            if p not in sys.path:
                sys.path.insert(0, p)
    import concourse  # noqa: F401


def _build_program():
    """Build + compile the Bass/Tile program (shared by all 8 cores)."""
    _ensure_path()
    from concourse import bacc, tile, mybir

    dt = mybir.dt
    nc = bacc.Bacc(
        "TRN2",
        target_bir_lowering=False,
        debug=False,
        enable_asserts=False,
        num_devices=N_CORES,
    )

    a_d = nc.dram_tensor("a_w", [NMID, 128, NCH * D], dt.float16, kind="ExternalInput").ap()
    xb_d = nc.dram_tensor("xb", [NMID + 1, 128, NSG * 2 * BL], dt.float16, kind="ExternalInput").ap()
    x0_d = nc.dram_tensor("x0", [P, BL], dt.float16, kind="ExternalInput").ap()
    pf_d = nc.dram_tensor("pf", [P, 128], dt.float16, kind="ExternalInput").ap()
    pl_d = nc.dram_tensor("pl", [128, NCH * O], dt.float16, kind="ExternalInput").ap()
    out_d = nc.dram_tensor("out", [O, BL], dt.float32, kind="ExternalOutput").ap()
    if NBG:
        x4_d = nc.dram_tensor("x4", [NMID + 1, 128, BL], dt.float16, kind="ExternalInput").ap()
        rep_d = nc.dram_tensor("rep", [128, ((2 * NBG + 3) // 4) * 128], dt.float16, kind="ExternalInput").ap()

    with tile.TileContext(nc) as tc:
        with tc.tile_pool(name="const", bufs=1) as cpool, \
             tc.tile_pool(name="aw", bufs=3) as apool, \
             tc.tile_pool(name="awd", bufs=2) as adpool, \
             tc.tile_pool(name="xbp", bufs=2) as xbpool, \
             tc.tile_pool(name="x4p", bufs=3) as x4pool, \
             tc.tile_pool(name="bcxp", bufs=2 * max(NBG, 1)) as bcxpool, \
             tc.tile_pool(name="vrp", bufs=2) as vrpool, \
             tc.tile_pool(name="up", bufs=10) as upool, \
             tc.tile_pool(name="misc", bufs=1) as mpool, \
             tc.tile_pool(name="pvp", bufs=2, space="PSUM") as pvpool, \
             tc.tile_pool(name="bcp", bufs=2, space="PSUM") as bcpool:

            # --- constants / per-core inputs resident in SBUF ---
            pf_sb = cpool.tile([P, 128], dt.float16, name="pf_sb")
            nc.sync.dma_start(out=pf_sb, in_=pf_d)
            pl_sb = cpool.tile([128, NCH * O], dt.float16, name="pl_sb")
            nc.sync.dma_start(out=pl_sb, in_=pl_d)
            x0_sb = cpool.tile([P, BL], dt.float16, name="x0_sb")
            nc.sync.dma_start(out=x0_sb, in_=x0_d)
            if NBG:
                rep_sb = cpool.tile([128, ((2 * NBG + 3) // 4) * 128], dt.float16, name="rep_sb")
                nc.sync.dma_start(out=rep_sb, in_=rep_d)

            # --- v0 = [psi_first^T | psi_first^T] @ x0 -> [v0 ; v0] ---
            pv_cur = pvpool.tile([128, 2 * TN], dt.float32, name="pv")
            for t in range(NT):
                nc.tensor.matmul(out=pv_cur[:, t * TN:(t + 1) * TN], lhsT=pf_sb,
                                 rhs=x0_sb[:, t * TN:(t + 1) * TN],
                                 start=True, stop=True)

            def emit_bcast(i):
                """PE replication matmuls producing bcast xb tiles for step i."""
                x4_sb = x4pool.tile([128, BL], dt.float16, name="x4_sb")
                nc.sync.dma_start(out=x4_sb, in_=x4_d[i])
                tiles = []
                for g in range(NBG):
                    bcx = bcxpool.tile([128, 2 * BL], dt.float16, name="bcx")
                    tiles.append(bcx)
                for j in range(2 * NBG):
                    g, q = j // 2, j % 2
                    if ROW_TILED:
                        k = j % 4
                    else:
                        k = 0
                    jq = j // 4 if ROW_TILED else 0
                    lhs = rep_sb[32 * k:32 * (k + 1), jq * 128:(jq + 1) * 128]
                    bps = bcpool.tile([128, 2 * TN], dt.float32, name="bps")
                    for t in range(NT):
                        nc.tensor.matmul(
                            out=bps[:, t * TN:(t + 1) * TN], lhsT=lhs,
                            rhs=x4_sb[32 * k:32 * (k + 1), t * TN:(t + 1) * TN],
                            start=True, stop=True,
                            tile_position=(32 * k, 0) if ROW_TILED else None)
                    nc.scalar.copy(
                        out=tiles[g][:, q * BL:(q + 1) * BL], in_=bps)
                return tiles

            bcx_cur = emit_bcast(0) if NBG else []

            for i in range(NMID + 1):
                last = (i == NMID)
                # evacuate [vT; vT] into v2 (scalar engine, PSUM -> SBUF fp16),
                # then column-duplicate on the vector engine (4x copy mode)
                v2 = vrpool.tile([128, 2 * BL], dt.float16, name="v2")
                nc.scalar.copy(out=v2[:, 0:BL], in_=pv_cur)
                nc.vector.tensor_copy(v2[:, BL:2 * BL], v2[:, 0:BL])

                # streamed x-broadcast for this step: two ~1.3MB DMAs
                xq_sb = xbpool.tile([128, NSG * 2 * BL], dt.float16, name="xq_sb")
                half = NSG * BL
                nc.sync.dma_start(out=xq_sb[:, :half], in_=xb_d[i, :, :half])
                nc.scalar.dma_start(out=xq_sb[:, half:], in_=xb_d[i, :, half:])

                if not last:
                    a_raw = apool.tile([128, NCH * D], dt.float16, name="a_raw")
                    nc.scalar.dma_start(out=a_raw, in_=a_d[i])
                    a_sb = adpool.tile([128, NCH * 128], dt.float16, name="a_sb")
                    av = a_sb.rearrange("p (c j e) -> p c j e", c=NCH, j=2, e=D)
                    ar = a_raw.rearrange("p (c e) -> p c e", c=NCH)
                    nc.scalar.copy(out=av[:, :, 0, :], in_=ar)
                    nc.scalar.copy(out=av[:, :, 1, :], in_=ar)
                    a_lhs = a_sb.rearrange("p (c m) -> p c m", c=NCH)
                    pv_nxt = pvpool.tile([128, 2 * TN], dt.float32, name="pv")
                else:
                    po = pvpool.tile([O, 2 * TN], dt.float32, name="pv")

                # u-products: us[g] = v2 * xb group tile (contiguous 2D, 2x mode)
                us = []
                for g in range(NG):
                    u2 = upool.tile([128, 2 * BL], dt.float16, name="u2")
                    if g < NSG:
                        xbg = xq_sb[:, g * 2 * BL:(g + 1) * 2 * BL]
                    else:
                        xbg = bcx_cur[g - NSG]
                    eng = nc.gpsimd if g in GP_GROUPS else nc.vector
                    eng.tensor_mul(u2, v2, xbg)
                    us.append(u2)

                # main contraction, chunk-outer (stationary reuse across t)
                out_ps = po if last else pv_nxt
                for c in range(NCH):
                    g, q = c // 2, c % 2
                    lhs = pl_sb[:, c * O:(c + 1) * O] if last else a_lhs[:, c]
                    for t in range(NT):
                        nc.tensor.matmul(
                            out=out_ps[:, t * TN:(t + 1) * TN],
                            lhsT=lhs,
                            rhs=us[g][:, q * BL + t * TN: q * BL + (t + 1) * TN],
                            start=(c == 0), stop=(c == NCH - 1))

                # broadcast xb tiles for step i+1 (PE + scalar evac)
                if NBG and not last:
                    bcx_cur = emit_bcast(i + 1)
                if not last:
                    pv_cur = pv_nxt

            # --- final scale by host-computed phi scalar (folded constant) ---
            out_sb = mpool.tile([O, BL], dt.float32, name="out_sb")
            sc_d = nc.dram_tensor("fscale", [1, 1], dt.float32, kind="ExternalInput").ap()
            sc_sb = mpool.tile([1, 1], dt.float32, name="sc_sb")
            nc.sync.dma_start(out=sc_sb, in_=sc_d)
            sc10 = mpool.tile([O, 1], dt.float32, name="sc10")
            nc.gpsimd.partition_broadcast(sc10, sc_sb)
            nc.scalar.mul(out=out_sb, in_=po, mul=sc10)
            nc.sync.dma_start(out=out_d, in_=out_sb)

    nc.compile()
    return nc


def _scale_schedule(x, psi_first, psi_mid, nsub=128):
    """Static per-step power-of-2 downscales keeping |v| in a small band."""
    xs = np.asarray(x[:nsub], np.float32)
    v = xs[:, 0] @ np.asarray(psi_first, np.float32).T
    ks = []
    for i in range(NMID):
        A = np.asarray(psi_mid[i], np.float32)            # [d, e, p]
        xi = xs[:, 1 + i]                                  # [s, p]
        u = np.einsum('sd,sp->sdp', v, xi).reshape(nsub, D * P)
        v = u @ A.transpose(0, 2, 1).reshape(D * P, D)
        vm = float(np.abs(v).max())
        k = 0
        while vm * 2.0 ** (-k) > VBAND:
            k += 1
        ks.append(k)
        v = v * 2.0 ** (-k)
    return ks


def kernel(x, psi_first, psi_mid, psi_last, phi_first, phi_mid, phi_last):
    _ensure_path()
    from concourse import bass_utils

    f16 = np.float16
    x = np.asarray(x, np.float32)
    psi_first = np.asarray(psi_first, np.float32)
    psi_mid = np.asarray(psi_mid, np.float32)
    psi_last = np.asarray(psi_last, np.float32)
    phi_first = np.asarray(phi_first, np.float64)
    phi_mid = np.asarray(phi_mid, np.float64)
    phi_last = np.asarray(phi_last, np.float64)

    if "nc" not in _cached:
        _cached["nc"] = _build_program()
    nc = _cached["nc"]

    ks = _scale_schedule(x, psi_first, psi_mid)

    # --- host phi chain: batch-independent scalar c ---
    w = phi_first[:, 0].copy()
    for i in range(NMID):
        w = w @ phi_mid[i, :, :, 1 + i]
    c_phi = float(w @ phi_last[:, Q - 1])

    # --- shared weight-side arrays ---
    scales = (2.0 ** -np.asarray(ks, np.float64)).astype(np.float32)
    # A2[i, r, e] = psi_mid[i, d(r), e, p(r)] * s_i  -> [62, 2048, 64]
    A2 = psi_mid.transpose(0, 1, 3, 2)[:, _D_IDX, _P_IDX, :]        # [62, 2048, 64]
    A2 = A2 * scales[:, None, None]
    A2c = A2.reshape(NMID, NCH, 128, D)
    a_host = np.ascontiguousarray(
        A2c.transpose(0, 2, 1, 3).reshape(NMID, 128, NCH * D)
    ).astype(f16)

    pf_host = np.concatenate([psi_first.T, psi_first.T], axis=1).astype(f16)  # [32, 128]

    # pl2[r, o] = psi_last[d(r), p(r), o] * 2^SH -> chunked [128, 16*O]
    pl2 = (psi_last * (2.0 ** SH_LAST))[_D_IDX, _P_IDX, :]          # [2048, O]
    pl_host = np.ascontiguousarray(
        pl2.reshape(NCH, 128, O).transpose(1, 0, 2).reshape(128, NCH * O)
    ).astype(f16)

    fscale_host = np.array(
        [[c_phi * 2.0 ** (sum(ks) - SH_LAST)]], dtype=np.float32)

    if NBG:
        # rep[32k + p, jq*128 + m] = 1 if p == 2*c + m//64 (c = global bcast chunk)
        nrq = ((2 * NBG + 3) // 4)
        rep_host = np.zeros((128, nrq * 128), f16)
        for j in range(2 * NBG):
            c = NCH - 2 * NBG + j
            k = (j % 4) if ROW_TILED else 0
            jq = (j // 4) if ROW_TILED else 0
            for m in range(128):
                p = 2 * c + m // 64
                rep_host[32 * k + p, jq * 128 + m] = 1.0

    # --- per-core batch shards ---
    xt = x.transpose(1, 2, 0).astype(f16)         # [Q, P, B]
    x0_all = xt[0]                                # [P, B]
    ridx = np.arange(128) // 64                   # [128] -> 0/1 within chunk
    in_maps = []
    for ci in range(N_CORES):
        sl = slice(ci * BL, (ci + 1) * BL)
        xs = np.ascontiguousarray(xt[1:, :, sl])            # [63, P, BL]
        # streamed groups: xbg[i, r, g*2BL + q*BL + s] = xs[i, 2*(2g+q)+r//64, s]
        xbg = np.empty((NMID + 1, 128, NSG, 2, BL), f16)
        for g in range(NSG):
            for q in range(2):
                xbg[:, :, g, q, :] = xs[:, 2 * (2 * g + q) + ridx, :]
        xbg = xbg.reshape(NMID + 1, 128, NSG * 2 * BL)
        m = {
            "a_w": a_host,
            "xb": np.ascontiguousarray(xbg),
            "x0": np.ascontiguousarray(x0_all[:, sl]),
            "pf": pf_host,
            "pl": pl_host,
            "fscale": fscale_host,
        }
        if NBG:
            m["x4"] = np.ascontiguousarray(np.tile(xs, (1, 4, 1)))   # [63, 128, BL]
            m["rep"] = rep_host
        in_maps.append(m)

    res = bass_utils.run_bass_kernel_spmd(nc, in_maps, core_ids=list(range(N_CORES)))
    _cached["in_maps"] = in_maps

    out = np.empty((B, O), np.float32)
    for ci in range(N_CORES):
        out[ci * BL:(ci + 1) * BL, :] = res.results[ci]["out"].T
    return out
